# revision 1
# baseline (speedup 1.0000x reference)
"""CrossAttention kernel for Trainium2, 8 NeuronCores, batch-parallel.

Problem (hardcoded): B=16, S=4096, D=1024; K=77, DE=768; H=16, Dh=64.
  q = hs @ Wq; k = ehs @ Wk; v = ehs @ Wv   (per-head attention, softmax over 77)
  out = concat_heads(softmax(q k^T / 8) v) @ Wo + bo

Sharding: data-parallel over batch — core c gets batches [2c, 2c+1]. No collectives.

Per-core dataflow (all big matmuls in float32r = full PE rate at free-dim>=256):
  - hs tiles are PE-transposed to hsT [D, s] so every GEMM contracts on partitions.
  - QT = Wq.T @ hsT (per 512-col s-tile), KT = Wk.T @ ehsT, V = ehs @ Wv (natural).
  - scoresT[j,s] = KT_h.T @ QT_h  (77x512 per head), exp on ACT,
    [V_h | ones(64)] stationary gives attn numerator + softmax colsums
    replicated on 64 partitions in one matmul; 1/den = exp(-ln(den)) on ACT
    (two table ops, vs DVE's 3.3us iterative reciprocal), one DVE multiply.
  - out[s,d] = attnT.T @ Wo + bo (natural row layout -> contiguous DMA out).
  - Software-pipelined one tile deep: PE runs next tile's transposes+QT over
    the softmax tail of the current tile so the PE p-state stays high.
"""

import numpy as np

import concourse.bass as bass
import concourse.mybir as mybir
from concourse.tile import TileContext
from concourse.bass_utils import run_bass_kernel_spmd
from concourse.masks import make_identity

# Problem constants
B, S, D = 16, 4096, 1024
KJ, DE = 77, 768
H, DH = 16, 64
INNER = H * DH  # 1024
NCORES = 8
BPC = B // NCORES  # batches per core = 2
ST = 512  # s-tile (columns of transposed activations)
NST = BPC * S // ST  # 16 s-tiles per core

F32 = mybir.dt.float32
F32R = mybir.dt.float32r
BF16 = mybir.dt.bfloat16

_CACHE = {}

# This walrus build allows at most ONE sync wait per instruction
# (setupSyncWait: "Too many sync wait commands"), but Tile freely attaches
# several (data-dep + queue credit + buffer WAR; the exit Drain carries one
# per engine/queue). Engines execute their streams in order, so hoisting all
# but one wait onto single-wait NoOps inserted just before the instruction
# is semantics-preserving. Applied at BIR-JSON level via to_json_bytes.
import orjson as _orjson


def _split_multiwait_bir(bir_bytes: bytes) -> bytes:
    bir = _orjson.loads(bir_bytes)
    changed = False
    for fn in bir.get("functions", []):
        for blk in fn.get("blocks", []):
            insts = blk.get("instructions", [])
            out = []
            for inst in insts:
                si = inst.get("sync_info")
                ow = (si or {}).get("on_wait") or []
                eng = inst.get("engine")
                if len(ow) > 1 and eng and eng != "Unassigned":
                    dbg = inst.get("debug", 0)
                    for j, w in enumerate(ow[:-1]):
                        out.append(
                            {
                                "name": f"{inst['name']}__sw{j}",
                                "opcode": "NoOp",
                                "engine": eng,
                                "ins": [],
                                "outs": [],
                                "debug": dbg,
                                "sync_info": {"on_wait": [w], "on_update": []},
                            }
                        )
                    si["on_wait"] = [ow[-1]]
                    changed = True
                out.append(inst)
            blk["instructions"] = out
    return _orjson.dumps(bir) if changed else bir_bytes


if not getattr(bass.Bass, "_multiwait_patched", False):
    _orig_to_json_bytes = bass.Bass.to_json_bytes

    def _patched_to_json_bytes(self):
        return _split_multiwait_bir(_orig_to_json_bytes(self))

    bass.Bass.to_json_bytes = _patched_to_json_bytes
    bass.Bass._multiwait_patched = True


def _r(ap):
    return ap.bitcast(F32R)


def _f(ap):
    return ap.bitcast(F32)


def build_bass():
    nc = bass.Bass(use_seq_codegen=True)

    hs_d = nc.dram_tensor("hs", [BPC, S, D], F32, kind="ExternalInput")
    ehs_d = nc.dram_tensor("ehs", [BPC, KJ, DE], F32, kind="ExternalInput")
    wq_d = nc.dram_tensor("wq", [D, INNER], F32, kind="ExternalInput")
    wk_d = nc.dram_tensor("wk", [DE, INNER], F32, kind="ExternalInput")
    wv_d = nc.dram_tensor("wv", [DE, INNER], F32, kind="ExternalInput")
    wo_d = nc.dram_tensor("wo", [INNER, D], F32, kind="ExternalInput")
    bo_d = nc.dram_tensor("bo", [D], F32, kind="ExternalInput")
    out_d = nc.dram_tensor("out", [BPC, S, D], F32, kind="ExternalOutput")

    with TileContext(nc) as tc:
        with (
            tc.tile_pool(name="const", bufs=1) as constp,
            tc.tile_pool(name="wq", bufs=8) as wqp,
            tc.tile_pool(name="wo", bufs=16) as wop,
            tc.tile_pool(name="wv", bufs=6) as wvp,
            tc.tile_pool(name="big4k", bufs=8) as big4k,
            tc.tile_pool(name="hsin", bufs=8) as hsinp,
            tc.tile_pool(name="hst", bufs=8) as hstp,
            tc.tile_pool(name="qt", bufs=8) as qtp,
            tc.tile_pool(name="att", bufs=8) as attp,
            tc.tile_pool(name="expp", bufs=3) as expp,
            tc.tile_pool(name="lnp", bufs=3) as lnp,
            tc.tile_pool(name="ps_big", bufs=2, space="PSUM") as ps_big,
            tc.tile_pool(name="ps_s", bufs=2, space="PSUM") as ps_s,
            tc.tile_pool(name="ps_o", bufs=3, space="PSUM") as ps_o,
        ):
            # ---- constants / weights ----
            ident = constp.tile([128, 128], F32, tag="ident")
            make_identity(nc, ident)
            # bf16 identity: bf16 PE transpose = 1.0 c/row vs f32's 2.0
            identb = constp.tile([128, 128], BF16, tag="identb")
            nc.vector.tensor_copy(identb, ident)

            bo_sb = constp.tile([128, D], F32, tag="bo")
            nc.gpsimd.dma_start(
                out=bo_sb, in_=bo_d[:].unsqueeze(0).to_broadcast((128, D))
            )

            wq_sb = []
            wo_sb = []
            for k in range(8):
                tq = big4k.tile([128, INNER], F32, tag="big4k", name=f"tq{k}")
                nc.gpsimd.dma_start(out=tq, in_=wq_d[k * 128:(k + 1) * 128, :])
                wqk = wqp.tile([128, INNER], BF16, tag="wq", name=f"wq{k}")
                nc.vector.tensor_copy(wqk, tq)
                wq_sb.append(wqk)
                to = big4k.tile([128, D], F32, tag="big4k", name=f"to{k}")
                nc.gpsimd.dma_start(out=to, in_=wo_d[k * 128:(k + 1) * 128, :])
                # split Wo into [128,512] halves: a strided moving operand
                # (512-of-1024 row pitch) runs the PE at half rate
                wokn = []
                for n in range(2):
                    w = wop.tile([128, 512], BF16, tag="wo", name=f"wo{k}_{n}")
                    nc.vector.tensor_copy(w, to[:, n * 512:(n + 1) * 512])
                    wokn.append(w)
                wo_sb.append(wokn)

            # ---- per-batch setup: ehsT, KT, V_ext ----
            kt_sb = [[None] * 8 for _ in range(BPC)]
            vext_sb = [None] * BPC
            for b in range(BPC):
                ehs_t = constp.tile([KJ, DE], F32, tag="ehs", bufs=2, name=f"ehs{b}")
                nc.gpsimd.dma_start(out=ehs_t, in_=ehs_d[b, :, :])

                ehsT = []
                for k in range(6):
                    pst = ps_o.tile([128, ST], F32, tag="ps_o", name=f"psT{b}_{k}")
                    nc.tensor.transpose(
                        pst[0:128, 0:KJ],
                        ehs_t[0:KJ, k * 128:(k + 1) * 128],
                        ident[0:KJ, 0:KJ],
                    )
                    et = constp.tile([128, KJ], F32R, tag=f"ehsT{k}", name=f"ehsT{b}_{k}")
                    nc.vector.tensor_copy(et, pst[0:128, 0:KJ])
                    ehsT.append(et)

                # Wk (shares big4k slots with Wv/hs_in/out tiles)
                wk_sb = []
                for k in range(6):
                    tk = big4k.tile([128, INNER], F32, tag="big4k", name=f"tk{b}_{k}")
                    nc.gpsimd.dma_start(out=tk, in_=wk_d[k * 128:(k + 1) * 128, :])
                    wkk = big4k.tile(
                        [128, INNER], F32R, tag="big4k", name=f"wk{b}_{k}"
                    )
                    nc.vector.tensor_copy(wkk, tk)
                    wk_sb.append(wkk)
                # KT[m] = (Wk block m).T @ ehsT  -> [128 inner, 77]
                # (plain f32: fp32r needs a moving free dim >= 256, KJ=77)
                for m in range(8):
                    pkt = ps_o.tile([128, ST], F32, tag="ps_o", name=f"pkt{b}_{m}")
                    for k in range(6):
                        nc.tensor.matmul(
                            pkt[:, 0:KJ],
                            _f(wk_sb[k][:, m * 128:(m + 1) * 128]),
                            _f(ehsT[k][:, 0:KJ]),
                            start=(k == 0),
                            stop=(k == 5),
                        )
                    ktm = constp.tile([128, KJ], F32R, tag=f"kt{b}_{m}", name=f"kt{b}_{m}")
                    nc.vector.tensor_copy(ktm, pkt[:, 0:KJ])
                    kt_sb[b][m] = ktm

                # Wv then V natural layout [77, inner], interleaved with ones cols
                tv_sb = []
                for k in range(6):
                    tv = big4k.tile([128, INNER], F32, tag="big4k", name=f"tv{b}_{k}")
                    nc.gpsimd.dma_start(out=tv, in_=wv_d[k * 128:(k + 1) * 128, :])
                    tv_sb.append(tv)
                # vext packs [V_h | ones(64)] per head: the ones block makes the
                # V-matmul emit the softmax colsums replicated on 64 partitions
                # (free on PE: cost scales with moving cols, not stationary
                # rows), so normalization needs no cross-partition broadcast.
                # Producers of fp32r-matmul inputs must round, so ones go in
                # via tensor_copy from a memset f32 tile, not memset directly.
                if b == 0:
                    ones64 = constp.tile([KJ, 64], F32, tag="ones64", name="ones64")
                    nc.gpsimd.memset(ones64, 1.0)
                vext = constp.tile([KJ, H * (DH + 64)], F32R, tag=f"vext{b}", name=f"vext{b}")
                for n in range(2):
                    wv_sb = []
                    for k in range(6):
                        w = wvp.tile([128, 512], F32R, tag="wv", name=f"wv{b}_{k}_{n}")
                        nc.vector.tensor_copy(w, tv_sb[k][:, n * 512:(n + 1) * 512])
                        wv_sb.append(w)
                    psv = ps_s.tile([KJ, 512], F32, tag="ps_s", name=f"psv{b}_{n}")
                    for k in range(6):
                        nc.tensor.matmul(
                            psv[0:KJ, :],
                            _r(ehsT[k][:, 0:KJ]),
                            _r(wv_sb[k]),
                            start=(k == 0),
                            stop=(k == 5),
                        )
                    for j in range(8):
                        h = n * 8 + j
                        nc.vector.tensor_copy(
                            vext[0:KJ, h * 128:h * 128 + 64],
                            psv[0:KJ, j * 64:(j + 1) * 64],
                        )
                        nc.vector.tensor_copy(
                            vext[0:KJ, h * 128 + 64:h * 128 + 128],
                            ones64,
                        )
                vext_sb[b] = vext

            # ---- main loop over s-tiles, software-pipelined one tile deep ----
            # PE emission order per iteration: heads(t) [scores run one head
            # ahead of attnV so ACT's exp is off the PE critical path], then
            # transposes+QT of tile t+1 (dense PE work covering the ln/exp/mul
            # tail of heads(t) — keeps the PE p-state high), then out(t).
            def emit_dma(t):
                b = t // (S // ST)
                s0 = (t % (S // ST)) * ST
                hs_in = []
                for r in range(4):
                    # gpsimd DMA casts f32->bf16 in flight
                    hin = hsinp.tile([128, D], BF16, tag="hsin", name=f"hsin{t}_{r}")
                    nc.gpsimd.dma_start(
                        out=hin, in_=hs_d[b, s0 + r * 128:s0 + (r + 1) * 128, :]
                    )
                    hs_in.append(hin)
                return hs_in

            def emit_transposes(t, hs_in):
                """PE-transpose hs tiles -> hsT for tile t. Emitted mid-way
                through the previous tile's heads loop: the PE ops fill the
                ACT-bound phase, and the DVE evictions land ahead of the
                remaining muls in the DVE queue so QT isn't head-of-line
                blocked behind the softmax chain."""
                hsT = []
                for k2 in range(4):
                    # bf16 halves PSUM footprint: two k-groups per bank
                    psx = ps_big.tile(
                        [128, 2 * ST], BF16, tag="ps_bigb", bufs=1, name=f"psx{t}_{k2}"
                    )
                    for j in range(2):
                        k = 2 * k2 + j
                        for r in range(4):
                            nc.tensor.transpose(
                                psx[:, j * ST + r * 128:j * ST + (r + 1) * 128],
                                hs_in[r][:, k * 128:(k + 1) * 128],
                                identb,
                            )
                    for j in range(2):
                        hk = hstp.tile(
                            [128, ST], BF16, tag="hst", name=f"hsT{t}_{2 * k2 + j}"
                        )
                        nc.vector.tensor_copy(hk, psx[:, j * ST:(j + 1) * ST])
                        hsT.append(hk)
                return hsT

            def emit_qt(t, hsT):
                qt = []
                for m in range(8):
                    psq = ps_big.tile([128, ST], F32, tag="ps_big", name=f"psq{t}_{m}")
                    for k in range(8):
                        nc.tensor.matmul(
                            psq,
                            wq_sb[k][:, m * 128:(m + 1) * 128],
                            hsT[k],
                            start=(k == 0),
                            stop=(k == 7),
                        )
                    qm = qtp.tile([128, ST], F32R, tag="qt", name=f"qt{t}_{m}")
                    nc.vector.tensor_copy(qm, psq)
                    qt.append(qm)
                return qt

            def emit_scores(t, b, h, qt):
                m, half = h // 2, h % 2
                prow = slice(half * 64, half * 64 + 64)
                pss = ps_s.tile([KJ, ST], F32, tag="ps_s", name=f"pss{t}_{h}")
                nc.tensor.matmul(
                    pss[0:KJ, :],
                    _r(kt_sb[b][m][prow, 0:KJ]),
                    _r(qt[m][prow, :]),
                    start=True,
                    stop=True,
                )
                ex = expp.tile([KJ, ST], F32R, tag="exp", name=f"exp{t}_{h}")
                nc.scalar.activation(
                    ex[0:KJ, :], pss[0:KJ, :], mybir.ActivationFunctionType.Exp
                )
                return ex

            def emit_attnv(t, b, h, ex, att):
                m, half = h // 2, h % 2
                prow = slice(half * 64, half * 64 + 64)
                pso = ps_o.tile([128, ST], F32, tag="ps_o", name=f"pso{t}_{h}")
                nc.tensor.matmul(
                    pso[:, :],
                    _r(vext_sb[b][0:KJ, h * 128:(h + 1) * 128]),
                    _r(ex[0:KJ, :]),
                    start=True,
                    stop=True,
                )
                # 1/den via exp(-ln(den)) on ACT over the replicated sums:
                # DVE's iterative reciprocal costs ~6.5ns per free-element per
                # lane (3.3us for 512 cols) regardless of partition count; two
                # 680ns ACT table ops replace it.
                lnd = lnp.tile([64, ST], F32, tag="lnd", name=f"lnd{t}_{h}")
                nc.scalar.activation(
                    lnd, pso[64:128, :], mybir.ActivationFunctionType.Ln
                )
                rb = lnp.tile([64, ST], F32, tag="rb", name=f"rb{t}_{h}")
                nc.scalar.activation(
                    rb, lnd, mybir.ActivationFunctionType.Exp, scale=-1.0
                )
                nc.vector.tensor_mul(att[m][prow, :], pso[0:64, :], rb)

            def emit_out(t, att):
                b = t // (S // ST)
                s0 = (t % (S // ST)) * ST
                for r in range(4):
                    ot = big4k.tile([128, D], F32, tag="big4k", name=f"out{t}_{r}")
                    for n in range(2):
                        pso2 = ps_big.tile(
                            [128, 512], F32, tag="ps_big", name=f"pso2{t}_{r}_{n}"
                        )
                        for k in range(8):
                            nc.tensor.matmul(
                                pso2,
                                att[k][:, r * 128:(r + 1) * 128],
                                wo_sb[k][n],
                                start=(k == 0),
                                stop=(k == 7),
                            )
                        nc.vector.tensor_add(
                            ot[:, n * 512:(n + 1) * 512],
                            pso2,
                            bo_sb[:, n * 512:(n + 1) * 512],
                        )
                    nc.gpsimd.dma_start(
                        out=out_d[b, s0 + r * 128:s0 + (r + 1) * 128, :], in_=ot
                    )

            qt = emit_qt(0, emit_transposes(0, emit_dma(0)))
            for t in range(NST):
                b = t // (S // ST)
                hs_in_next = emit_dma(t + 1) if t + 1 < NST else None
                att = [
                    attp.tile([128, ST], BF16, tag="att", name=f"att{t}_{m}")
                    for m in range(8)
                ]
                hsT_next = None
                exs = [emit_scores(t, b, 0, qt)]
                for h in range(H):
                    if h + 1 < H:
                        exs.append(emit_scores(t, b, h + 1, qt))
                    emit_attnv(t, b, h, exs[h], att)
                    if h == 8 and t + 1 < NST:
                        hsT_next = emit_transposes(t + 1, hs_in_next)
                qt = emit_qt(t + 1, hsT_next) if t + 1 < NST else None
                emit_out(t, att)

    return nc


def kernel_jax(hidden_states, encoder_hidden_states, Wq, Wk, Wv, Wo, bo, **unused):
    """Batch-parallel cross-attention on 8 NeuronCores via the PJRT backend.

    Core c computes batches [2c, 2c+1]; outputs are concatenated on host.
    """
    import jax
    import jax.numpy as jnp

    if "jfn" not in _CACHE:

        def _f(hs, ehs, wq, wk, wv, wo, bo_):
            q = hs @ wq
            k = ehs @ wk
            v = ehs @ wv
            bpc, s, _ = hs.shape
            kj = ehs.shape[1]
            q = q.reshape(bpc, s, H, DH).transpose(0, 2, 1, 3)
            k = k.reshape(bpc, kj, H, DH).transpose(0, 2, 1, 3)
            v = v.reshape(bpc, kj, H, DH).transpose(0, 2, 1, 3)
            scores = jnp.einsum("bhsd,bhkd->bhsk", q, k) * (1.0 / np.sqrt(DH))
            probs = jax.nn.softmax(scores, axis=-1)
            out = jnp.einsum("bhsk,bhkd->bhsd", probs, v)
            out = out.transpose(0, 2, 1, 3).reshape(bpc, s, H * DH)
            return out @ wo + bo_

        _CACHE["jfn"] = jax.jit(_f)

    jfn = _CACHE["jfn"]
    devs = jax.devices()[:NCORES]
    hs = np.asarray(hidden_states, dtype=np.float32)
    ehs = np.asarray(encoder_hidden_states, dtype=np.float32)
    consts = [
        np.asarray(x, dtype=np.float32) for x in (Wq, Wk, Wv, Wo, bo)
    ]

    outs = []
    for c, d in enumerate(devs):
        args = [
            jax.device_put(np.ascontiguousarray(hs[c * BPC:(c + 1) * BPC]), d),
            jax.device_put(np.ascontiguousarray(ehs[c * BPC:(c + 1) * BPC]), d),
        ] + [jax.device_put(x, d) for x in consts]
        outs.append(jfn(*args))
    return np.concatenate([np.asarray(o) for o in outs], axis=0)


def kernel(hidden_states, encoder_hidden_states, Wq, Wk, Wv, Wo, bo, **unused):

    if "nc" not in _CACHE:
        _CACHE["nc"] = build_bass()
    nc = _CACHE["nc"]

    wq_scaled = (np.asarray(Wq, dtype=np.float32) * (1.0 / np.sqrt(DH))).astype(
        np.float32
    )
    wk = np.ascontiguousarray(np.asarray(Wk, dtype=np.float32))
    wv = np.ascontiguousarray(np.asarray(Wv, dtype=np.float32))
    wo = np.ascontiguousarray(np.asarray(Wo, dtype=np.float32))
    bo = np.ascontiguousarray(np.asarray(bo, dtype=np.float32))
    hs = np.asarray(hidden_states, dtype=np.float32)
    ehs = np.asarray(encoder_hidden_states, dtype=np.float32)

    in_maps = []
    for c in range(NCORES):
        in_maps.append(
            {
                "hs": np.ascontiguousarray(hs[c * BPC:(c + 1) * BPC]),
                "ehs": np.ascontiguousarray(ehs[c * BPC:(c + 1) * BPC]),
                "wq": wq_scaled,
                "wk": wk,
                "wv": wv,
                "wo": wo,
                "bo": bo,
            }
        )

    res = run_bass_kernel_spmd(nc, in_maps, list(range(NCORES)))
    outs = [res.results[c]["out"] for c in range(NCORES)]
    return np.concatenate(outs, axis=0)



# revision 27
# speedup vs baseline: 1.0953x; 1.0953x over previous
"""CrossAttention kernel for Trainium2, 8 NeuronCores, batch-parallel.

Problem (hardcoded): B=16, S=4096, D=1024; K=77, DE=768; H=16, Dh=64.
  q = hs @ Wq; k = ehs @ Wk; v = ehs @ Wv   (per-head attention, softmax over 77)
  out = concat_heads(softmax(q k^T / 8) v) @ Wo + bo

Sharding: data-parallel over batch - core c gets batches [2c, 2c+1]. No collectives.

Per-core dataflow (all matmuls bf16 -> full PE rate, 1 cycle/moving-col):
  - hs tiles are PE-transposed to hsT [D, s] so every GEMM contracts on partitions.
  - QT = Wq.T @ hsT (per 512-col s-tile), KT = Wk.T @ ehsT, V = ehs @ Wv (natural).
  - scoresT[j,s] = KT_h.T @ QT_h (77x512 per head). The two heads of an m-pair
    use contraction rows 0:64 / 64:128 -> row-tiled (tile_position auto-derived)
    and emitted back-to-back so the PE runs them concurrently.
  - exp on ACT, [V_h | ones(64)] stationary gives attn numerator + softmax
    colsums replicated on 64 partitions in one matmul; 1/den = exp(-ln(den)) on
    ACT (two table ops from the shared ln+exp table, vs DVE's 3.3us iterative
    reciprocal), one DVE multiply.
  - out[s,d] = attnT.T @ Wo + bo (natural row layout -> contiguous DMA out).
  - Software-pipelined one tile deep: PE runs next tile's transposes+QT over
    the softmax tail of the current tile so the PE p-state stays high.
  - Weight/ehs DMAs ride the ACT and DVE queues in first-use order so the PE
    isn't head-of-line blocked behind 8MB of weights at startup; hs/out use
    the gpsimd queue. Batch-1 KT/V setup is emitted inside tile 0's
    ACT-bound attention phase to fill PE gaps.
"""

import numpy as np

import concourse.bass as bass
import concourse.mybir as mybir
from concourse.tile import TileContext
from concourse.bass_utils import run_bass_kernel_spmd
from concourse.masks import make_identity

# Problem constants
B, S, D = 16, 4096, 1024
KJ, DE = 77, 768
H, DH = 16, 64
INNER = H * DH  # 1024
NCORES = 8
BPC = B // NCORES  # batches per core = 2
ST = 512  # s-tile (columns of transposed activations)
NST = BPC * S // ST  # 16 s-tiles per core

F32 = mybir.dt.float32
BF16 = mybir.dt.bfloat16

_CACHE = {}

# This walrus build allows at most ONE sync wait per instruction
# (setupSyncWait: "Too many sync wait commands"), but Tile freely attaches
# several (data-dep + queue credit + buffer WAR; the exit Drain carries one
# per engine/queue). Engines execute their streams in order, so hoisting all
# but one wait onto single-wait NoOps inserted just before the instruction
# is semantics-preserving. Applied at BIR-JSON level via to_json_bytes.
import orjson as _orjson


def _split_multiwait_bir(bir_bytes: bytes) -> bytes:
    bir = _orjson.loads(bir_bytes)
    changed = False
    for fn in bir.get("functions", []):
        for blk in fn.get("blocks", []):
            insts = blk.get("instructions", [])
            out = []
            for inst in insts:
                si = inst.get("sync_info")
                ow = (si or {}).get("on_wait") or []
                eng = inst.get("engine")
                if len(ow) > 1 and eng and eng != "Unassigned":
                    dbg = inst.get("debug", 0)
                    for j, w in enumerate(ow[:-1]):
                        out.append(
                            {
                                "name": f"{inst['name']}__sw{j}",
                                "opcode": "NoOp",
                                "engine": eng,
                                "ins": [],
                                "outs": [],
                                "debug": dbg,
                                "sync_info": {"on_wait": [w], "on_update": []},
                            }
                        )
                    si["on_wait"] = [ow[-1]]
                    changed = True
                out.append(inst)
            blk["instructions"] = out
    return _orjson.dumps(bir) if changed else bir_bytes


if not getattr(bass.Bass, "_multiwait_patched", False):
    _orig_to_json_bytes = bass.Bass.to_json_bytes

    def _patched_to_json_bytes(self):
        return _split_multiwait_bir(_orig_to_json_bytes(self))

    bass.Bass.to_json_bytes = _patched_to_json_bytes
    bass.Bass._multiwait_patched = True


def build_bass():
    nc = bass.Bass(use_seq_codegen=True)

    hs_d = nc.dram_tensor("hs", [BPC, S, D], F32, kind="ExternalInput")
    ehs_d = nc.dram_tensor("ehs", [BPC, KJ, DE], F32, kind="ExternalInput")
    wq_d = nc.dram_tensor("wq", [D, INNER], F32, kind="ExternalInput")
    wk_d = nc.dram_tensor("wk", [DE, INNER], F32, kind="ExternalInput")
    wv_d = nc.dram_tensor("wv", [DE, INNER], F32, kind="ExternalInput")
    wo_d = nc.dram_tensor("wo", [INNER, D], F32, kind="ExternalInput")
    bo_d = nc.dram_tensor("bo", [D], F32, kind="ExternalInput")
    out_d = nc.dram_tensor("out", [BPC, S, D], F32, kind="ExternalOutput")

    EXP = mybir.ActivationFunctionType.Exp
    LN = mybir.ActivationFunctionType.Ln

    with TileContext(nc) as tc:
        with (
            tc.tile_pool(name="const", bufs=1) as constp,
            tc.tile_pool(name="wq", bufs=8) as wqp,
            tc.tile_pool(name="wo", bufs=16) as wop,
            tc.tile_pool(name="wk", bufs=6) as wkp,
            tc.tile_pool(name="wv", bufs=12) as wvp,
            tc.tile_pool(name="outp", bufs=8) as outp,
            tc.tile_pool(name="hsin", bufs=8) as hsinp,
            tc.tile_pool(name="hst", bufs=8) as hstp,
            tc.tile_pool(name="qt", bufs=8) as qtp,
            tc.tile_pool(name="att", bufs=16) as attp,
            tc.tile_pool(name="expp", bufs=5) as expp,
            tc.tile_pool(name="lnp", bufs=4) as lnp,
            tc.tile_pool(name="wqs", bufs=4) as wqsp,
            tc.tile_pool(name="ps_big", bufs=2, space="PSUM") as ps_big,
            tc.tile_pool(name="ps_s", bufs=4, space="PSUM") as ps_s,
            tc.tile_pool(name="ps_o", bufs=2, space="PSUM") as ps_o,
        ):
            # ---- constants ----
            # bf16 identity: bf16 PE transpose = 1.0 c/row vs f32's 2.0
            identb = constp.tile([128, 128], BF16, tag="identb")
            make_identity(nc, identb)
            ones64 = constp.tile([KJ, 64], BF16, tag="ones64")
            nc.gpsimd.memset(ones64, 1.0)

            # ---- DMA queues, in first-use order ----
            # gpsimd queue: hs tiles in + out tiles out (streaming)
            def emit_dma(t):
                b = t // (S // ST)
                s0 = (t % (S // ST)) * ST
                hs_in = []
                for r in range(4):
                    # gpsimd DMA casts f32->bf16 in flight
                    hin = hsinp.tile([128, D], BF16, tag="hsin", name=f"hsin{t}_{r}")
                    nc.gpsimd.dma_start(
                        out=hin, in_=hs_d[b, s0 + r * 128:s0 + (r + 1) * 128, :]
                    )
                    hs_in.append(hin)
                return hs_in

            # Casting DMAs must use the gpsimd queue; emit in first-use order
            # so the PE isn't head-of-line blocked behind later weights.
            ehs_sb = []
            for b in range(BPC):
                e = constp.tile([KJ, DE], BF16, tag=f"ehs{b}", name=f"ehs{b}")
                nc.gpsimd.dma_start(out=e, in_=ehs_d[b, :, :])
                ehs_sb.append(e)

            hs_in0 = emit_dma(0)

            # Wq rides the idle sync DMA queue as raw f32 (casting DMAs are
            # gpsimd-only) + DVE cast, so QT(0) isn't gated on the gpsimd
            # queue draining ehs/hs/Wk first.
            wq_sb = []
            for k in range(8):
                stg = wqsp.tile([128, INNER], F32, tag="wqs", name=f"wqs{k}")
                nc.sync.dma_start(out=stg, in_=wq_d[k * 128:(k + 1) * 128, :])
                w = wqp.tile([128, INNER], BF16, tag="wq", name=f"wq{k}")
                nc.vector.tensor_copy(w, stg)
                wq_sb.append(w)
            wk_sb = []
            for k in range(6):
                w = wkp.tile([128, INNER], BF16, tag="wk", name=f"wk{k}")
                nc.gpsimd.dma_start(out=w, in_=wk_d[k * 128:(k + 1) * 128, :])
                wk_sb.append(w)
            wv_sb = [[None] * 2 for _ in range(6)]
            for k in range(6):
                for n in range(2):
                    w = wvp.tile([128, 512], BF16, tag="wv", name=f"wv{k}_{n}")
                    nc.gpsimd.dma_start(
                        out=w, in_=wv_d[k * 128:(k + 1) * 128, n * 512:(n + 1) * 512]
                    )
                    wv_sb[k][n] = w

            # Wo tiles are DMA'd inside tile 0 (first used at emit_out(0));
            # [128,512] tiles: a strided moving operand runs the PE at half
            # rate, so Wo halves live in separate unstrided tiles.
            wo_sb = [[None] * 2 for _ in range(8)]

            def emit_wo_dmas():
                for k in range(8):
                    for n in range(2):
                        w = wop.tile([128, 512], BF16, tag="wo", name=f"wo{k}_{n}")
                        nc.gpsimd.dma_start(
                            out=w,
                            in_=wo_d[k * 128:(k + 1) * 128, n * 512:(n + 1) * 512],
                        )
                        wo_sb[k][n] = w

            # non-casting DMA: ride the idle sync queue
            bo_sb = constp.tile([128, D], F32, tag="bo")
            nc.sync.dma_start(
                out=bo_sb, in_=bo_d[:].unsqueeze(0).to_broadcast((128, D))
            )

            # ---- per-batch setup pieces (PE + DVE only, no DMA) ----
            ehsT_sb = [None] * BPC
            kt_sb = [[None] * 8 for _ in range(BPC)]
            vext_sb = [None] * BPC

            def emit_ehst(b):
                # bf16 transposes must land in a bf16 PSUM view; borrow a
                # ps_big slot via bitcast and pack all 6 (6*128 cols <= 1024)
                pst = ps_big.tile(
                    [128, ST], F32, tag="ps_big", name=f"psT{b}"
                ).bitcast(BF16)
                for k in range(6):
                    nc.tensor.transpose(
                        pst[0:128, k * 128:k * 128 + KJ],
                        ehs_sb[b][0:KJ, k * 128:(k + 1) * 128],
                        identb[0:KJ, 0:KJ],
                    )
                ehsT = []
                for k in range(6):
                    et = constp.tile(
                        [128, KJ], BF16, tag=f"ehsT{k}", bufs=2, name=f"ehsT{b}_{k}"
                    )
                    nc.vector.tensor_copy(et, pst[0:128, k * 128:k * 128 + KJ])
                    ehsT.append(et)
                ehsT_sb[b] = ehsT

            def emit_kt(b):
                # KT[m] = (Wk block m).T @ ehsT  -> [128 inner, 77]
                ehsT = ehsT_sb[b]
                for m in range(8):
                    pkt = ps_big.tile([128, ST], F32, tag="ps_big", name=f"pkt{b}_{m}")
                    for k in range(6):
                        nc.tensor.matmul(
                            pkt[:, 0:KJ],
                            wk_sb[k][:, m * 128:(m + 1) * 128],
                            ehsT[k][:, 0:KJ],
                            start=(k == 0),
                            stop=(k == 5),
                        )
                    ktm = constp.tile([128, KJ], BF16, tag=f"kt{b}_{m}", name=f"kt{b}_{m}")
                    nc.vector.tensor_copy(ktm, pkt[:, 0:KJ])
                    kt_sb[b][m] = ktm

            def emit_v(b):
                # V natural [77, inner] via psum, packed into vext with ones:
                # [V_h | ones(64)] per head - the ones block makes the V-matmul
                # emit the softmax colsums replicated on 64 partitions for free
                # (stationary cols are free on PE; cost scales with moving cols).
                ehsT = ehsT_sb[b]
                vext = constp.tile(
                    [KJ, H * (DH + 64)], BF16, tag=f"vext{b}", name=f"vext{b}"
                )
                for n in range(2):
                    psv = ps_big.tile([KJ, ST], F32, tag="ps_big", name=f"psv{b}_{n}")
                    for k in range(6):
                        nc.tensor.matmul(
                            psv[0:KJ, :],
                            ehsT[k][:, 0:KJ],
                            wv_sb[k][n],
                            start=(k == 0),
                            stop=(k == 5),
                        )
                    for j in range(8):
                        h = n * 8 + j
                        nc.vector.tensor_copy(
                            vext[0:KJ, h * 128:h * 128 + 64],
                            psv[0:KJ, j * 64:(j + 1) * 64],
                        )
                        nc.vector.tensor_copy(
                            vext[0:KJ, h * 128 + 64:h * 128 + 128],
                            ones64,
                        )
                vext_sb[b] = vext

            # ---- per-tile emitters ----
            def emit_transpose_group(t, hs_in, k2, hsT):
                """PE-transpose 2 of 8 hs k-groups -> hsT for tile t. The 4
                groups are spread across the previous tile's heads loop so
                each ps_x (bufs=1) reuse has a full pair-cadence for its DVE
                evictions to land - emitting them en bloc stalled the PE ~1us
                per tile on the ring's WAR wait."""
                # bf16 halves PSUM footprint: two k-groups per bank. The
                # scratch borrows a ps_big slot (bitcast f32->bf16 view): that
                # ring is idle mid-tile, its bufs=2 gives the evictions a
                # group of slack, and the freed bank funds ps_s bufs=4 so
                # scores pairs stop stalling on exp-gated ring reuse.
                psx = ps_big.tile(
                    [128, ST], F32, tag="ps_big", name=f"psx{t}_{k2}"
                ).bitcast(BF16)
                for j in range(2):
                    k = 2 * k2 + j
                    for r in range(4):
                        nc.tensor.transpose(
                            psx[:, j * ST + r * 128:j * ST + (r + 1) * 128],
                            hs_in[r][:, k * 128:(k + 1) * 128],
                            identb,
                        )
                for j in range(2):
                    hk = hstp.tile(
                        [128, ST], BF16, tag="hst", name=f"hsT{t}_{2 * k2 + j}"
                    )
                    nc.vector.tensor_copy(hk, psx[:, j * ST:(j + 1) * ST])
                    hsT.append(hk)

            def emit_transposes(t, hs_in):
                hsT = []
                for k2 in range(4):
                    emit_transpose_group(t, hs_in, k2, hsT)
                return hsT

            def emit_qt(t, hsT):
                qt = []
                for m in range(8):
                    psq = ps_big.tile([128, ST], F32, tag="ps_big", name=f"psq{t}_{m}")
                    for k in range(8):
                        nc.tensor.matmul(
                            psq,
                            wq_sb[k][:, m * 128:(m + 1) * 128],
                            hsT[k],
                            start=(k == 0),
                            stop=(k == 7),
                        )
                    qm = qtp.tile([128, ST], BF16, tag="qt", name=f"qt{t}_{m}")
                    nc.vector.tensor_copy(qm, psq)
                    qt.append(qm)
                return qt

            def emit_spair(t, b, m, qt):
                """Scores for heads 2m / 2m+1: contraction rows 0:64 / 64:128,
                emitted back-to-back -> row-tiled concurrent on the PE."""
                exs = []
                pss = []
                for half in range(2):
                    prow = slice(half * 64, half * 64 + 64)
                    p = ps_s.tile([KJ, ST], F32, tag="ps_s", name=f"pss{t}_{2*m+half}")
                    nc.tensor.matmul(
                        p[0:KJ, :],
                        kt_sb[b][m][prow, 0:KJ],
                        qt[m][prow, :],
                        start=True,
                        stop=True,
                    )
                    pss.append(p)
                for half in range(2):
                    ex = expp.tile([KJ, ST], BF16, tag="exp", name=f"exp{t}_{2*m+half}")
                    nc.scalar.activation(ex[0:KJ, :], pss[half][0:KJ, :], EXP)
                    exs.append(ex)
                return exs

            def emit_apost(t, b, m, exs, att):
                for half in range(2):
                    h = 2 * m + half
                    prow = slice(half * 64, half * 64 + 64)
                    pso = ps_o.tile([128, ST], F32, tag="ps_o", name=f"pso{t}_{h}")
                    nc.tensor.matmul(
                        pso[:, :],
                        vext_sb[b][0:KJ, h * 128:(h + 1) * 128],
                        exs[half][0:KJ, :],
                        start=True,
                        stop=True,
                    )
                    # 1/den via exp(-ln(den)) on ACT over the replicated sums
                    lnd = lnp.tile([64, ST], F32, tag="lnd", name=f"lnd{t}_{h}")
                    nc.scalar.activation(lnd, pso[64:128, :], LN)
                    rb = lnp.tile([64, ST], F32, tag="rb", name=f"rb{t}_{h}")
                    nc.scalar.activation(rb, lnd, EXP, scale=-1.0)
                    nc.vector.tensor_mul(att[m][prow, :], pso[0:64, :], rb)

            def emit_out(t, att, rs=(0, 1, 2, 3)):
                b = t // (S // ST)
                s0 = (t % (S // ST)) * ST
                for r in rs:
                    ot = outp.tile([128, D], F32, tag="outp", name=f"out{t}_{r}")
                    for n in range(2):
                        pso2 = ps_big.tile(
                            [128, 512], F32, tag="ps_big", name=f"pso2{t}_{r}_{n}"
                        )
                        for k in range(8):
                            nc.tensor.matmul(
                                pso2,
                                att[k][:, r * 128:(r + 1) * 128],
                                wo_sb[k][n],
                                start=(k == 0),
                                stop=(k == 7),
                            )
                        nc.vector.tensor_add(
                            ot[:, n * 512:(n + 1) * 512],
                            pso2,
                            bo_sb[:, n * 512:(n + 1) * 512],
                        )
                    nc.gpsimd.dma_start(
                        out=out_d[b, s0 + r * 128:s0 + (r + 1) * 128, :], in_=ot
                    )

            # ---- prologue: batch-0 setup interleaved with tile-0 prefetch ----
            emit_ehst(0)
            emit_ehst(1)
            hsT0 = emit_transposes(0, hs_in0)
            emit_v(0)
            qt = emit_qt(0, hsT0)
            emit_kt(0)

            # ---- main loop over s-tiles, software-pipelined one tile deep ----
            for t in range(NST):
                b = t // (S // ST)
                hs_in_next = emit_dma(t + 1) if t + 1 < NST else None
                if t == 0:
                    emit_wo_dmas()
                att = [
                    attp.tile([128, ST], BF16, tag="att", name=f"att{t}_{m}")
                    for m in range(8)
                ]
                hsT_next = None
                exs = [emit_spair(t, b, 0, qt)]
                for m in range(8):
                    if m + 1 < 8:
                        exs.append(emit_spair(t, b, m + 1, qt))
                    emit_apost(t, b, m, exs[m], att)
                    if m == 4 and t + 1 < NST:
                        hsT_next = emit_transposes(t + 1, hs_in_next)
                    if t == 0 and m == 2:
                        emit_kt(1)
                    if t == 0 and m == 6:
                        emit_v(1)
                    # last tile has no next-tile transposes/QT to keep the PE
                    # warm through the softmax tail: feed it the deferred half
                    # of out(NST-2) instead
                    if t == NST - 1 and m == 2:
                        emit_out(t - 1, att_prev, rs=(2,))
                    if t == NST - 1 and m == 5:
                        emit_out(t - 1, att_prev, rs=(3,))
                qt = emit_qt(t + 1, hsT_next) if t + 1 < NST else None
                if t == NST - 2:
                    emit_out(t, att, rs=(0, 1))
                    att_prev = att
                else:
                    emit_out(t, att)

    return nc


def kernel(hidden_states, encoder_hidden_states, Wq, Wk, Wv, Wo, bo, **unused):

    if "nc" not in _CACHE:
        _CACHE["nc"] = build_bass()
    nc = _CACHE["nc"]

    wq_scaled = (np.asarray(Wq, dtype=np.float32) * (1.0 / np.sqrt(DH))).astype(
        np.float32
    )
    wk = np.ascontiguousarray(np.asarray(Wk, dtype=np.float32))
    wv = np.ascontiguousarray(np.asarray(Wv, dtype=np.float32))
    wo = np.ascontiguousarray(np.asarray(Wo, dtype=np.float32))
    bo = np.ascontiguousarray(np.asarray(bo, dtype=np.float32))
    hs = np.asarray(hidden_states, dtype=np.float32)
    ehs = np.asarray(encoder_hidden_states, dtype=np.float32)

    in_maps = []
    for c in range(NCORES):
        in_maps.append(
            {
                "hs": np.ascontiguousarray(hs[c * BPC:(c + 1) * BPC]),
                "ehs": np.ascontiguousarray(ehs[c * BPC:(c + 1) * BPC]),
                "wq": wq_scaled,
                "wk": wk,
                "wv": wv,
                "wo": wo,
                "bo": bo,
            }
        )

    res = run_bass_kernel_spmd(nc, in_maps, list(range(NCORES)))
    outs = [res.results[c]["out"] for c in range(NCORES)]
    return np.concatenate(outs, axis=0)


# revision 30
# speedup vs baseline: 1.1291x; 1.0309x over previous
"""CrossAttention kernel for Trainium2, 8 NeuronCores, batch-parallel.

Problem (hardcoded): B=16, S=4096, D=1024; K=77, DE=768; H=16, Dh=64.
  q = hs @ Wq; k = ehs @ Wk; v = ehs @ Wv   (per-head attention, softmax over 77)
  out = concat_heads(softmax(q k^T / 8) v) @ Wo + bo

Sharding: data-parallel over batch - core c gets batches [2c, 2c+1]. No collectives.

Per-core dataflow (all matmuls bf16 -> full PE rate, 1 cycle/moving-col):
  - hs tiles are PE-transposed to hsT [D, s] so every GEMM contracts on partitions.
  - QT = Wq.T @ hsT (per 512-col s-tile), KT = Wk.T @ ehsT, V = ehs @ Wv (natural).
  - scoresT[j,s] = KT_h.T @ QT_h (77x512 per head). The two heads of an m-pair
    use contraction rows 0:64 / 64:128 -> row-tiled (tile_position auto-derived)
    and emitted back-to-back so the PE runs them concurrently.
  - exp on ACT, [V_h | ones(64)] stationary gives attn numerator + softmax
    colsums replicated on 64 partitions in one matmul; 1/den = exp(-ln(den)) on
    ACT (two table ops from the shared ln+exp table, vs DVE's 3.3us iterative
    reciprocal), one DVE multiply.
  - out[s,d] = attnT.T @ Wo + bo (natural row layout -> contiguous DMA out).
  - Software-pipelined one tile deep: PE runs next tile's transposes+QT over
    the softmax tail of the current tile so the PE p-state stays high.
  - Weight/ehs DMAs ride the ACT and DVE queues in first-use order so the PE
    isn't head-of-line blocked behind 8MB of weights at startup; hs/out use
    the gpsimd queue. Batch-1 KT/V setup is emitted inside tile 0's
    ACT-bound attention phase to fill PE gaps.
"""

import numpy as np

import concourse.bass as bass
import concourse.mybir as mybir
from concourse.tile import TileContext
from concourse.bass_utils import run_bass_kernel_spmd
from concourse.masks import make_identity

# Problem constants
B, S, D = 16, 4096, 1024
KJ, DE = 77, 768
H, DH = 16, 64
INNER = H * DH  # 1024
NCORES = 8
BPC = B // NCORES  # batches per core = 2
ST = 512  # s-tile (columns of transposed activations)
NST = BPC * S // ST  # 16 s-tiles per core

F32 = mybir.dt.float32
BF16 = mybir.dt.bfloat16

_CACHE = {}

# This walrus build allows at most ONE sync wait per instruction
# (setupSyncWait: "Too many sync wait commands"), but Tile freely attaches
# several (data-dep + queue credit + buffer WAR; the exit Drain carries one
# per engine/queue). Engines execute their streams in order, so hoisting all
# but one wait onto single-wait NoOps inserted just before the instruction
# is semantics-preserving. Applied at BIR-JSON level via to_json_bytes.
import orjson as _orjson


def _split_multiwait_bir(bir_bytes: bytes) -> bytes:
    bir = _orjson.loads(bir_bytes)
    changed = False
    for fn in bir.get("functions", []):
        for blk in fn.get("blocks", []):
            insts = blk.get("instructions", [])
            out = []
            for inst in insts:
                si = inst.get("sync_info")
                ow = (si or {}).get("on_wait") or []
                eng = inst.get("engine")
                if len(ow) > 1 and eng and eng != "Unassigned":
                    dbg = inst.get("debug", 0)
                    for j, w in enumerate(ow[:-1]):
                        out.append(
                            {
                                "name": f"{inst['name']}__sw{j}",
                                "opcode": "NoOp",
                                "engine": eng,
                                "ins": [],
                                "outs": [],
                                "debug": dbg,
                                "sync_info": {"on_wait": [w], "on_update": []},
                            }
                        )
                    si["on_wait"] = [ow[-1]]
                    changed = True
                out.append(inst)
            blk["instructions"] = out
    return _orjson.dumps(bir) if changed else bir_bytes


if not getattr(bass.Bass, "_multiwait_patched", False):
    _orig_to_json_bytes = bass.Bass.to_json_bytes

    def _patched_to_json_bytes(self):
        return _split_multiwait_bir(_orig_to_json_bytes(self))

    bass.Bass.to_json_bytes = _patched_to_json_bytes
    bass.Bass._multiwait_patched = True


def build_bass():
    nc = bass.Bass(use_seq_codegen=True)

    hs_d = nc.dram_tensor("hs", [BPC, S, D], F32, kind="ExternalInput")
    ehs_d = nc.dram_tensor("ehs", [BPC, KJ, DE], F32, kind="ExternalInput")
    wq_d = nc.dram_tensor("wq", [D, INNER], F32, kind="ExternalInput")
    wk_d = nc.dram_tensor("wk", [DE, INNER], F32, kind="ExternalInput")
    wv_d = nc.dram_tensor("wv", [DE, INNER], F32, kind="ExternalInput")
    wo_d = nc.dram_tensor("wo", [INNER, D], F32, kind="ExternalInput")
    bo_d = nc.dram_tensor("bo", [D], F32, kind="ExternalInput")
    out_d = nc.dram_tensor("out", [BPC, S, D], F32, kind="ExternalOutput")

    EXP = mybir.ActivationFunctionType.Exp
    LN = mybir.ActivationFunctionType.Ln

    with TileContext(nc) as tc:
        with (
            tc.tile_pool(name="const", bufs=1) as constp,
            tc.tile_pool(name="wq", bufs=8) as wqp,
            tc.tile_pool(name="wo", bufs=16) as wop,
            tc.tile_pool(name="wk", bufs=6) as wkp,
            tc.tile_pool(name="wv", bufs=12) as wvp,
            tc.tile_pool(name="outp", bufs=8) as outp,
            tc.tile_pool(name="hsin", bufs=8) as hsinp,
            tc.tile_pool(name="hst", bufs=8) as hstp,
            tc.tile_pool(name="qt", bufs=8) as qtp,
            tc.tile_pool(name="att", bufs=16) as attp,
            tc.tile_pool(name="expp", bufs=5) as expp,
            tc.tile_pool(name="lnp", bufs=4) as lnp,
            tc.tile_pool(name="wqs", bufs=4) as wqsp,
            tc.tile_pool(name="ps_x", bufs=1, space="PSUM") as ps_x,
            tc.tile_pool(name="ps_big", bufs=2, space="PSUM") as ps_big,
            tc.tile_pool(name="ps_s", bufs=3, space="PSUM") as ps_s,
            tc.tile_pool(name="ps_o", bufs=2, space="PSUM") as ps_o,
        ):
            # ---- constants ----
            # bf16 identity: bf16 PE transpose = 1.0 c/row vs f32's 2.0
            identb = constp.tile([128, 128], BF16, tag="identb")
            make_identity(nc, identb)
            ones64 = constp.tile([KJ, 64], BF16, tag="ones64")
            nc.gpsimd.memset(ones64, 1.0)

            # ---- DMA queues, in first-use order ----
            # gpsimd queue: hs tiles in + out tiles out (streaming)
            def emit_dma(t):
                b = t // (S // ST)
                s0 = (t % (S // ST)) * ST
                hs_in = []
                for r in range(4):
                    # gpsimd DMA casts f32->bf16 in flight
                    hin = hsinp.tile([128, D], BF16, tag="hsin", name=f"hsin{t}_{r}")
                    nc.gpsimd.dma_start(
                        out=hin, in_=hs_d[b, s0 + r * 128:s0 + (r + 1) * 128, :]
                    )
                    hs_in.append(hin)
                return hs_in

            # Casting DMAs must use the gpsimd queue; emit in first-use order
            # so the PE isn't head-of-line blocked behind later weights.
            ehs_sb = []
            for b in range(BPC):
                e = constp.tile([KJ, DE], BF16, tag=f"ehs{b}", name=f"ehs{b}")
                nc.gpsimd.dma_start(out=e, in_=ehs_d[b, :, :])
                ehs_sb.append(e)

            hs_in0 = emit_dma(0)

            # Wq rides the idle sync DMA queue as raw f32 (casting DMAs are
            # gpsimd-only) + DVE cast, so QT(0) isn't gated on the gpsimd
            # queue draining ehs/hs/Wk first.
            wq_sb = []
            for k in range(8):
                stg = wqsp.tile([128, INNER], F32, tag="wqs", name=f"wqs{k}")
                nc.sync.dma_start(out=stg, in_=wq_d[k * 128:(k + 1) * 128, :])
                w = wqp.tile([128, INNER], BF16, tag="wq", name=f"wq{k}")
                nc.vector.tensor_copy(w, stg)
                wq_sb.append(w)
            wk_sb = []
            for k in range(6):
                w = wkp.tile([128, INNER], BF16, tag="wk", name=f"wk{k}")
                nc.gpsimd.dma_start(out=w, in_=wk_d[k * 128:(k + 1) * 128, :])
                wk_sb.append(w)
            wv_sb = [[None] * 2 for _ in range(6)]
            for k in range(6):
                for n in range(2):
                    w = wvp.tile([128, 512], BF16, tag="wv", name=f"wv{k}_{n}")
                    nc.gpsimd.dma_start(
                        out=w, in_=wv_d[k * 128:(k + 1) * 128, n * 512:(n + 1) * 512]
                    )
                    wv_sb[k][n] = w

            # Wo tiles are DMA'd inside tile 0 (first used at emit_out(0));
            # [128,512] tiles: a strided moving operand runs the PE at half
            # rate, so Wo halves live in separate unstrided tiles.
            wo_sb = [[None] * 2 for _ in range(8)]

            def emit_wo_dmas():
                for k in range(8):
                    for n in range(2):
                        w = wop.tile([128, 512], BF16, tag="wo", name=f"wo{k}_{n}")
                        nc.gpsimd.dma_start(
                            out=w,
                            in_=wo_d[k * 128:(k + 1) * 128, n * 512:(n + 1) * 512],
                        )
                        wo_sb[k][n] = w

            # non-casting DMA: ride the idle sync queue
            bo_sb = constp.tile([128, D], F32, tag="bo")
            nc.sync.dma_start(
                out=bo_sb, in_=bo_d[:].unsqueeze(0).to_broadcast((128, D))
            )

            # ---- per-batch setup pieces (PE + DVE only, no DMA) ----
            ehsT_sb = [None] * BPC
            kt_sb = [[None] * 8 for _ in range(BPC)]
            vext_sb = [None] * BPC

            def emit_ehst(b):
                # bf16 transposes must land in a bf16 PSUM tile; pack all 6
                # into one ps_x slot (6*128 cols <= 1024)
                pst = ps_x.tile([128, 2 * ST], BF16, tag="ps_x", name=f"psT{b}")
                for k in range(6):
                    nc.tensor.transpose(
                        pst[0:128, k * 128:k * 128 + KJ],
                        ehs_sb[b][0:KJ, k * 128:(k + 1) * 128],
                        identb[0:KJ, 0:KJ],
                    )
                ehsT = []
                for k in range(6):
                    et = constp.tile(
                        [128, KJ], BF16, tag=f"ehsT{k}", bufs=2, name=f"ehsT{b}_{k}"
                    )
                    nc.vector.tensor_copy(et, pst[0:128, k * 128:k * 128 + KJ])
                    ehsT.append(et)
                ehsT_sb[b] = ehsT

            def emit_kt(b):
                # KT[m] = (Wk block m).T @ ehsT  -> [128 inner, 77]
                ehsT = ehsT_sb[b]
                for m in range(8):
                    pkt = ps_big.tile([128, ST], F32, tag="ps_big", name=f"pkt{b}_{m}")
                    for k in range(6):
                        nc.tensor.matmul(
                            pkt[:, 0:KJ],
                            wk_sb[k][:, m * 128:(m + 1) * 128],
                            ehsT[k][:, 0:KJ],
                            start=(k == 0),
                            stop=(k == 5),
                        )
                    ktm = constp.tile([128, KJ], BF16, tag=f"kt{b}_{m}", name=f"kt{b}_{m}")
                    nc.vector.tensor_copy(ktm, pkt[:, 0:KJ])
                    kt_sb[b][m] = ktm

            def emit_v(b):
                # V natural [77, inner] via psum, packed into vext with ones:
                # [V_h | ones(64)] per head - the ones block makes the V-matmul
                # emit the softmax colsums replicated on 64 partitions for free
                # (stationary cols are free on PE; cost scales with moving cols).
                ehsT = ehsT_sb[b]
                vext = constp.tile(
                    [KJ, H * (DH + 64)], BF16, tag=f"vext{b}", name=f"vext{b}"
                )
                for n in range(2):
                    psv = ps_big.tile([KJ, ST], F32, tag="ps_big", name=f"psv{b}_{n}")
                    for k in range(6):
                        nc.tensor.matmul(
                            psv[0:KJ, :],
                            ehsT[k][:, 0:KJ],
                            wv_sb[k][n],
                            start=(k == 0),
                            stop=(k == 5),
                        )
                    for j in range(8):
                        h = n * 8 + j
                        nc.vector.tensor_copy(
                            vext[0:KJ, h * 128:h * 128 + 64],
                            psv[0:KJ, j * 64:(j + 1) * 64],
                        )
                        nc.vector.tensor_copy(
                            vext[0:KJ, h * 128 + 64:h * 128 + 128],
                            ones64,
                        )
                vext_sb[b] = vext

            # ---- per-tile emitters ----
            def emit_transpose_group(t, hs_in, k2, hsT):
                """PE-transpose 2 of 8 hs k-groups -> hsT for tile t. The 4
                groups are spread across the previous tile's heads loop so
                each ps_x (bufs=1) reuse has a full pair-cadence for its DVE
                evictions to land - emitting them en bloc stalled the PE ~1us
                per tile on the ring's WAR wait."""
                # bf16 halves PSUM footprint: two k-groups per bank
                psx = ps_x.tile([128, 2 * ST], BF16, tag="ps_x", name=f"psx{t}_{k2}")
                for j in range(2):
                    k = 2 * k2 + j
                    for r in range(4):
                        nc.tensor.transpose(
                            psx[:, j * ST + r * 128:j * ST + (r + 1) * 128],
                            hs_in[r][:, k * 128:(k + 1) * 128],
                            identb,
                        )
                for j in range(2):
                    hk = hstp.tile(
                        [128, ST], BF16, tag="hst", name=f"hsT{t}_{2 * k2 + j}"
                    )
                    nc.vector.tensor_copy(hk, psx[:, j * ST:(j + 1) * ST])
                    hsT.append(hk)

            def emit_transposes(t, hs_in):
                hsT = []
                for k2 in range(4):
                    emit_transpose_group(t, hs_in, k2, hsT)
                return hsT

            def emit_qt(t, hsT):
                qt = []
                for m in range(8):
                    psq = ps_big.tile([128, ST], F32, tag="ps_big", name=f"psq{t}_{m}")
                    for k in range(8):
                        nc.tensor.matmul(
                            psq,
                            wq_sb[k][:, m * 128:(m + 1) * 128],
                            hsT[k],
                            start=(k == 0),
                            stop=(k == 7),
                        )
                    qm = qtp.tile([128, ST], BF16, tag="qt", name=f"qt{t}_{m}")
                    nc.vector.tensor_copy(qm, psq)
                    qt.append(qm)
                return qt

            def emit_spair(t, b, m, qt):
                """Scores for heads 2m / 2m+1: contraction rows 0:64 / 64:128,
                emitted back-to-back -> row-tiled concurrent on the PE."""
                exs = []
                pss = []
                for half in range(2):
                    prow = slice(half * 64, half * 64 + 64)
                    p = ps_s.tile([KJ, ST], F32, tag="ps_s", name=f"pss{t}_{2*m+half}")
                    nc.tensor.matmul(
                        p[0:KJ, :],
                        kt_sb[b][m][prow, 0:KJ],
                        qt[m][prow, :],
                        start=True,
                        stop=True,
                    )
                    pss.append(p)
                for half in range(2):
                    ex = expp.tile([KJ, ST], BF16, tag="exp", name=f"exp{t}_{2*m+half}")
                    nc.scalar.activation(ex[0:KJ, :], pss[half][0:KJ, :], EXP)
                    exs.append(ex)
                return exs

            def emit_apost(t, b, m, exs, att):
                for half in range(2):
                    h = 2 * m + half
                    prow = slice(half * 64, half * 64 + 64)
                    pso = ps_o.tile([128, ST], F32, tag="ps_o", name=f"pso{t}_{h}")
                    nc.tensor.matmul(
                        pso[:, :],
                        vext_sb[b][0:KJ, h * 128:(h + 1) * 128],
                        exs[half][0:KJ, :],
                        start=True,
                        stop=True,
                    )
                    # 1/den via exp(-ln(den)) on ACT over the replicated sums
                    lnd = lnp.tile([64, ST], F32, tag="lnd", name=f"lnd{t}_{h}")
                    nc.scalar.activation(lnd, pso[64:128, :], LN)
                    rb = lnp.tile([64, ST], F32, tag="rb", name=f"rb{t}_{h}")
                    nc.scalar.activation(rb, lnd, EXP, scale=-1.0)
                    nc.vector.tensor_mul(att[m][prow, :], pso[0:64, :], rb)

            def emit_out(t, att, rs=(0, 1, 2, 3), split_dma=False):
                b = t // (S // ST)
                s0 = (t % (S // ST)) * ST
                for r in rs:
                    ot = outp.tile([128, D], F32, tag="outp", name=f"out{t}_{r}")
                    for n in range(2):
                        pso2 = ps_big.tile(
                            [128, 512], F32, tag="ps_big", name=f"pso2{t}_{r}_{n}"
                        )
                        for k in range(8):
                            nc.tensor.matmul(
                                pso2,
                                att[k][:, r * 128:(r + 1) * 128],
                                wo_sb[k][n],
                                start=(k == 0),
                                stop=(k == 7),
                            )
                        nc.vector.tensor_add(
                            ot[:, n * 512:(n + 1) * 512],
                            pso2,
                            bo_sb[:, n * 512:(n + 1) * 512],
                        )
                        if split_dma:
                            # last tile: DMA each half as soon as its add
                            # lands so the drain overlaps the final adds
                            nc.gpsimd.dma_start(
                                out=out_d[
                                    b,
                                    s0 + r * 128:s0 + (r + 1) * 128,
                                    n * 512:(n + 1) * 512,
                                ],
                                in_=ot[:, n * 512:(n + 1) * 512],
                            )
                    if not split_dma:
                        nc.gpsimd.dma_start(
                            out=out_d[b, s0 + r * 128:s0 + (r + 1) * 128, :], in_=ot
                        )

            # ---- prologue: batch-0 setup interleaved with tile-0 prefetch ----
            emit_ehst(0)
            emit_ehst(1)
            hsT0 = emit_transposes(0, hs_in0)
            emit_v(0)
            qt = emit_qt(0, hsT0)
            emit_kt(0)

            # ---- main loop over s-tiles, software-pipelined one tile deep ----
            for t in range(NST):
                b = t // (S // ST)
                hs_in_next = emit_dma(t + 1) if t + 1 < NST else None
                if t == 0:
                    emit_wo_dmas()
                att = [
                    attp.tile([128, ST], BF16, tag="att", name=f"att{t}_{m}")
                    for m in range(8)
                ]
                hsT_next = None
                exs = [emit_spair(t, b, 0, qt)]
                for m in range(8):
                    if m + 1 < 8:
                        exs.append(emit_spair(t, b, m + 1, qt))
                    emit_apost(t, b, m, exs[m], att)
                    if m == 4 and t + 1 < NST:
                        hsT_next = emit_transposes(t + 1, hs_in_next)
                    if t == 0 and m == 2:
                        emit_kt(1)
                    if t == 0 and m == 6:
                        emit_v(1)
                    # last tile has no next-tile transposes/QT to keep the PE
                    # warm through the softmax tail: feed it three deferred
                    # row-chunks of out(NST-2) instead
                    if t == NST - 1 and m == 1:
                        emit_out(t - 1, att_prev, rs=(1,))
                    if t == NST - 1 and m == 3:
                        emit_out(t - 1, att_prev, rs=(2,))
                    if t == NST - 1 and m == 5:
                        emit_out(t - 1, att_prev, rs=(3,))
                qt = emit_qt(t + 1, hsT_next) if t + 1 < NST else None
                if t == NST - 2:
                    emit_out(t, att, rs=(0,))
                    att_prev = att
                elif t == NST - 1:
                    emit_out(t, att, split_dma=True)
                else:
                    emit_out(t, att)

    return nc


def kernel(hidden_states, encoder_hidden_states, Wq, Wk, Wv, Wo, bo, **unused):

    if "nc" not in _CACHE:
        _CACHE["nc"] = build_bass()
    nc = _CACHE["nc"]

    wq_scaled = (np.asarray(Wq, dtype=np.float32) * (1.0 / np.sqrt(DH))).astype(
        np.float32
    )
    wk = np.ascontiguousarray(np.asarray(Wk, dtype=np.float32))
    wv = np.ascontiguousarray(np.asarray(Wv, dtype=np.float32))
    wo = np.ascontiguousarray(np.asarray(Wo, dtype=np.float32))
    bo = np.ascontiguousarray(np.asarray(bo, dtype=np.float32))
    hs = np.asarray(hidden_states, dtype=np.float32)
    ehs = np.asarray(encoder_hidden_states, dtype=np.float32)

    in_maps = []
    for c in range(NCORES):
        in_maps.append(
            {
                "hs": np.ascontiguousarray(hs[c * BPC:(c + 1) * BPC]),
                "ehs": np.ascontiguousarray(ehs[c * BPC:(c + 1) * BPC]),
                "wq": wq_scaled,
                "wk": wk,
                "wv": wv,
                "wo": wo,
                "bo": bo,
            }
        )

    res = run_bass_kernel_spmd(nc, in_maps, list(range(NCORES)))
    outs = [res.results[c]["out"] for c in range(NCORES)]
    return np.concatenate(outs, axis=0)
